# revision 24
# baseline (speedup 1.0000x reference)
"""Trainium2 Bass kernel for nn_DR_CML (data-parallel over batch, 8 cores).

Math: xm[b,i,j] = x[b,i]*lm_w[j] + lm_b[j], so every loo row is affine in
the scalar s[b,i] = xbar[b] - x[b,i]/xd, and the [B,K,xd-1] diff tensor
collapses to S[b,k] = 511*pos[b,k] + q0[b] + q1[b]*y + q2[b]*y^2 with
q_j[b] = sum_i phi_j(s[b,i]) for smooth scalar functions phi_j.

This kernel approximates each phi_j by a per-call Chebyshev polynomial
(deg 2) over the observed s-range, so q_j reduces to power sums
T_k = sum_i t_i^k of t = -x/512 combined with xbar via host-folded
per-partition coefficient rows.  The positive branch (mu/lv MLPs of a
single scalar v = xbar or s_last) is likewise fitted with deg-4 polys
per partition-half, eliminating every tanh/exp/relu from the critical
path.  Fit error is ~1e-5; final f32 rel err ~1e-4 (tolerance 2e-2).

Layout: x repacked [2*(B/8), xd/2] = [128, 256]; y duplicated to both
partition halves so the final masked matmul P = F2^T @ R2 sums the
halves, masks, and propensity weights in one PE op per core.  Host sums
the 8 [4,33] tiles and applies the closed-form combine.
"""
import math

import numpy as np

B, XD, K, H = 512, 512, 32, 7
NCORES = 8
BL = B // NCORES          # 64 rows per core
HC = XD // 2              # 256 columns after repack
DEGQ = 2                  # phi_j poly degree (power sums T1..T_DEGQ)
DEGG = 4                  # positive-branch g poly degree
NB = 8                    # basis cols: 1, xb, xb2, T1, T2, xb*T1, tr, tr*xb
NG = DEGG + 1             # v-power cols: 1, v, v2, v3, v4
LN2 = math.log(2.0)

_prog_cache = {}
_last = None              # (nc, in_maps) from the most recent kernel() call

# feature flags (HW bring-up bisection)
USE_TTR = False            # tensor_tensor_reduce vs TT + tensor_reduce
DENSE_MM1 = False         # copy T1 to a dense tile before the MM1 moving
CONSTS_ON_GPSIMD = False  # consts DMA on gpsimd queue instead of scalar


def _fold_consts(p):
    """Fold linear_map + MLP weights into scalar-MLP coefficients (f64)."""
    lm_w = p['lm_w'].astype(np.float64)
    lm_b = p['lm_b'].astype(np.float64)
    c = lm_b * (XD - 1) / XD

    def fold(w1, b1):
        u = lm_w @ w1.astype(np.float64)
        v_base = lm_b @ w1.astype(np.float64) + b1.astype(np.float64)
        v_c = c @ w1.astype(np.float64) + b1.astype(np.float64)
        return u, v_base, v_c

    u_mu, vb_mu, vc_mu = fold(p['mu_w1'], p['mu_b1'])
    u_lv, vb_lv, vc_lv = fold(p['lv_w1'], p['lv_b1'])
    u_mun, _, vc_mun = fold(p['mun_w1'], p['mun_b1'])
    u_lvn, _, vc_lvn = fold(p['lvn_w1'], p['lvn_b1'])

    return {
        'u_mu': u_mu, 'vb_mu': vb_mu, 'vc_mu': vc_mu,
        'u_lv': u_lv, 'vb_lv': vb_lv, 'vc_lv': vc_lv,
        'u_mun': u_mun, 'vc_mun': vc_mun,
        'u_lvn': u_lvn, 'vc_lvn': vc_lvn,
        'w2_mu': p['mu_w2'][:, 0].astype(np.float64),
        'w2_lv': p['lv_w2'][:, 0].astype(np.float64),
        'w2_mun': p['mun_w2'][:, 0].astype(np.float64),
        'w2_lvn': p['lvn_w2'][:, 0].astype(np.float64),
        'b2_mu': float(p['mu_b2'][0]), 'b2_lv': float(p['lv_b2'][0]),
        'b2_mun': float(p['mun_b2'][0]), 'b2_lvn': float(p['lvn_b2'][0]),
        'ps_b': float(p['ps_b'][0]),
    }


def _fit_poly(f, lo, hi, deg):
    if hi - lo < 1e-12:
        hi = lo + 1e-6
    c = np.polynomial.chebyshev.Chebyshev.interpolate(f, deg, domain=[lo, hi])
    out = np.zeros(deg + 1)
    cc = c.convert(kind=np.polynomial.Polynomial).coef
    out[:len(cc)] = cc
    return out


def _design(fc, x):
    """Per-call poly fits + coefficient rows (all f64 host math)."""
    x64 = x.astype(np.float64)
    xbar = x64.mean(1)
    s = xbar[:, None] - x64[:, :XD - 1] / XD
    smin, smax = float(s.min()), float(s.max())

    def mlp_scalar(v, u, vb, w2, b2):
        return np.maximum(np.multiply.outer(v, u) + vb, 0.0) @ w2 + b2

    def phi(sv, idx):
        mun = mlp_scalar(sv, fc['u_mun'], fc['vc_mun'], fc['w2_mun'],
                         fc['b2_mun'])
        lvn = np.tanh(mlp_scalar(sv, fc['u_lvn'], fc['vc_lvn'],
                                 fc['w2_lvn'], fc['b2_lvn']))
        ev = np.exp(-lvn - LN2)
        if idx == 0:
            return ev * mun * mun + 0.5 * lvn
        if idx == 1:
            return -2.0 * ev * mun
        return ev

    qcoef = [_fit_poly(lambda t: phi(t, j), smin, smax, DEGQ)
             for j in range(3)]

    # basis cols: [1, xb, xb2, T1, T2, xb*T1, tr, tr*xb]
    def coef_rows(upper):
        R = np.zeros((3, NB))
        for j in range(3):
            for d in range(DEGQ + 1):
                c = qcoef[j][d]
                if c == 0.0:
                    continue
                for m in range(d + 1):
                    k = d - m
                    w = c * math.comb(d, m)
                    if k == 0:
                        R[j, m] += w * float(HC)           # Traw_0 = 256
                    elif k == 1:
                        R[j, [3, 5][m]] += w               # T1, xb*T1
                    elif k == 2:
                        assert m == 0
                        R[j, 4] += w                       # T2
                if upper:  # exclude the i=511 (treat) slot
                    for m in range(d + 1):
                        k = d - m
                        w = -c * math.comb(d, m) * (-1.0 / XD) ** k
                        if k == 0:
                            R[j, m] += w                   # -xb^d term
                        else:
                            R[j, 6 + m] += w               # tr * xb^m
        return R

    Rlow, Rup = coef_rows(False), coef_rows(True)

    slast = s[:, -1]
    vlo_l, vhi_l = float(xbar.min()), float(xbar.max())
    vlo_u, vhi_u = float(slast.min()), float(slast.max())

    def g_funcs(vb_mu, vb_lv):
        mu = lambda v: mlp_scalar(v, fc['u_mu'], vb_mu, fc['w2_mu'],
                                  fc['b2_mu'])
        lv = lambda v: np.tanh(mlp_scalar(v, fc['u_lv'], vb_lv,
                                          fc['w2_lv'], fc['b2_lv']))
        return (mu,
                lambda v: (XD - 1) * np.exp(-lv(v)) * 0.5,
                lambda v: (XD - 1) * lv(v) * 0.5)

    gcoef_l = [_fit_poly(g, vlo_l, vhi_l, DEGG)
               for g in g_funcs(fc['vb_mu'], fc['vb_lv'])]
    gcoef_u = [_fit_poly(g, vlo_u, vhi_u, DEGG)
               for g in g_funcs(fc['vc_mu'], fc['vc_lv'])]

    return {'Rlow': Rlow, 'Rup': Rup,
            'gcoef_l': gcoef_l, 'gcoef_u': gcoef_u,
            'ps_b': fc['ps_b']}


# ---- consts tensor layout
C_M = 0                    # cols 0:128   M'' = -M pair-exchange matrix
C_Q = 128                  # cols 128:152 three [128, NB] q-coef blocks
C_G = C_Q + 3 * NB         # cols 152:167 three [128, NG] g-coef blocks
C_W = C_G + 3 * NG         # total width


def _build_program(dsg):
    from contextlib import ExitStack
    import concourse.tile as tile
    from concourse import bacc, mybir

    f32 = mybir.dt.float32
    Alu = mybir.AluOpType
    Act = mybir.ActivationFunctionType
    Ax = mybir.AxisListType

    nc = bacc.Bacc("TRN2", target_bir_lowering=False, debug=False,
                   num_devices=NCORES)

    xt_d = nc.dram_tensor("xt", [2 * BL, HC], f32, kind="ExternalInput").ap()
    y2_d = nc.dram_tensor("y2", [2 * BL, K], f32, kind="ExternalInput").ap()
    psw_d = nc.dram_tensor("psw", [2 * BL, HC], f32,
                           kind="ExternalInput").ap()
    tc_d = nc.dram_tensor("consts", [128, C_W], f32,
                          kind="ExternalInput").ap()
    out_d = nc.dram_tensor("out", [4, K + 1], f32, kind="ExternalOutput").ap()

    with tile.TileContext(nc) as tcx, ExitStack() as ctx:
        sb = ctx.enter_context(tcx.tile_pool(name="sb", bufs=1))
        ps = ctx.enter_context(tcx.tile_pool(name="ps", bufs=1, space="PSUM"))

        # ---- DMAs, one per queue so descriptor gen overlaps
        tx = sb.tile([128, HC], f32, tag="tx")
        nc.sync.dma_start(tx[:], xt_d)
        tc = sb.tile([128, C_W], f32, tag="tc")
        if CONSTS_ON_GPSIMD:
            nc.gpsimd.dma_start(tc[:], tc_d)
        else:
            nc.scalar.dma_start(tc[:], tc_d)
        tpsw = sb.tile([128, HC], f32, tag="tpsw")
        nc.gpsimd.dma_start(tpsw[:], psw_d)
        ty = sb.tile([128, K], f32, tag="ty")
        nc.sync.dma_start(ty[:], y2_d)

        # hoist the ACT table load before data arrives (for the one exp)
        warm = sb.tile([1, 1], f32, tag="warm")
        nc.scalar.activation(warm[:], nc.const_aps.tensor(0.0, (1, 1)),
                             Act.Exp, bias=0.0, scale=1.0)

        Mpp = tc[:, C_M:C_M + 128]

        # ---- tiles
        bas = sb.tile([128, NB], f32, tag="bas")
        vb = sb.tile([128, NG], f32, tag="vb")
        st = sb.tile([128, 2], f32, tag="st")      # [tr-stage | rp]
        qh = sb.tile([128, 3], f32, tag="qh")
        gt = sb.tile([128, 3], f32, tag="gt")      # g2, H1, H3
        gt2 = sb.tile([128, 3], f32, tag="gt2")    # gg, gg*g2, q0-gg*g2
        w0t = sb.tile([128, 1], f32, tag="w0t")
        w1t = sb.tile([128, 1], f32, tag="w1t")
        qx = sb.tile([128, 3], f32, tag="qx")
        qmix = sb.tile([128, 3], f32, tag="qmix")
        qflip = sb.tile([128, 3], f32, tag="qflip")
        yt2 = sb.tile([128, K], f32, tag="yt2")
        R2 = sb.tile([128, K + 1], f32, tag="R2")
        F2 = sb.tile([128, 4], f32, tag="F2")
        S1 = sb.tile([128, K], f32, tag="S1")
        c0a = sb.tile([128, 1], f32, tag="c0a")
        eprt = sb.tile([128, 1], f32, tag="eprt")
        numt = sb.tile([128, 1], f32, tag="numt")
        den0 = sb.tile([128, 1], f32, tag="den0")
        den1 = sb.tile([128, 1], f32, tag="den1")
        r0 = sb.tile([128, 1], f32, tag="r0")
        r1 = sb.tile([128, 1], f32, tag="r1")
        f0t = sb.tile([128, 1], f32, tag="f0t")
        f1t = sb.tile([128, 1], f32, tag="f1t")
        texs = sb.tile([128, 1], f32, tag="texs")
        t = sb.tile([128, HC], f32, tag="t")
        junk = sb.tile([128, HC], f32, tag="junk")
        junk2 = sb.tile([128, HC], f32, tag="junk2")
        junkq = sb.tile([128, NB], f32, tag="junkq")
        junkg = sb.tile([128, NG], f32, tag="junkg")
        outs = sb.tile([4, K + 1], f32, tag="outs")

        xbex = ps.tile([128, 1], f32, tag="xbex")
        fex = ps.tile([128, 2], f32, tag="fex")
        P = ps.tile([4, K + 1], f32, tag="P")

        # ---- gpsimd: early memsets (no data deps)
        nc.gpsimd.memset(bas[:, 0:1], 1.0)
        nc.gpsimd.memset(bas[0:BL, 6:8], 0.0)
        nc.gpsimd.memset(vb[:, 0:1], 1.0)
        nc.gpsimd.memset(R2[0:BL, K:K + 1], 1.0)
        nc.gpsimd.memset(R2[BL:128, K:K + 1], 0.0)
        nc.gpsimd.memset(st[0:BL, 0:1], 0.0)

        # ---- DVE spine
        nc.vector.tensor_scalar(t[:], tx[:], -1.0 / XD, None, Alu.mult)
        if USE_TTR:
            nc.vector.tensor_tensor_reduce(junk[:], tx[:], tpsw[:], 1.0, 0.0,
                                           Alu.mult, Alu.add,
                                           accum_out=st[:, 1:2])
            nc.vector.tensor_reduce(bas[:, 3:4], t[:], Ax.X, Alu.add)
            nc.vector.tensor_tensor_reduce(junk2[:], t[:], t[:], 1.0, 0.0,
                                           Alu.mult, Alu.add,
                                           accum_out=bas[:, 4:5])
        else:
            nc.vector.tensor_tensor(junk[:], tx[:], tpsw[:], Alu.mult)
            nc.vector.tensor_reduce(st[:, 1:2], junk[:], Ax.X, Alu.add)
            nc.vector.tensor_reduce(bas[:, 3:4], t[:], Ax.X, Alu.add)
            nc.vector.tensor_tensor(junk2[:], t[:], t[:], Alu.mult)
            nc.vector.tensor_reduce(bas[:, 4:5], junk2[:], Ax.X, Alu.add)

        # ---- gpsimd: treat staging (needs tx)
        nc.gpsimd.tensor_copy(st[BL:128, 0:1], tx[BL:128, HC - 1:HC])
        nc.gpsimd.tensor_copy(bas[BL:128, 6:7], tx[BL:128, HC - 1:HC])

        # ---- PE: exchanges
        if DENSE_MM1:
            T1d = sb.tile([128, 1], f32, tag="T1d")
            nc.vector.tensor_copy(T1d[:], bas[:, 3:4])
            nc.tensor.matmul(xbex[:], Mpp, T1d[:], start=True, stop=True)
        else:
            nc.tensor.matmul(xbex[:], Mpp, bas[:, 3:4], start=True, stop=True)
        nc.tensor.matmul(fex[:], Mpp, st[:], start=True, stop=True)

        # ---- basis completion (DVE)
        nc.vector.tensor_copy(bas[:, 1:2], xbex[:])
        nc.vector.tensor_tensor(bas[:, 2:3], bas[:, 1:2], bas[:, 1:2],
                                Alu.mult)
        nc.vector.tensor_tensor(bas[:, 5:6], bas[:, 1:2], bas[:, 3:4],
                                Alu.mult)
        nc.vector.tensor_tensor(bas[:, 7:8], bas[:, 6:7], bas[:, 1:2],
                                Alu.mult)

        # ---- v powers (gpsimd)
        nc.gpsimd.tensor_copy(vb[0:BL, 1:2], bas[0:BL, 1:2])
        nc.vector.tensor_scalar(vb[BL:128, 1:2], tx[BL:128, HC - 2:HC - 1],
                                -1.0 / XD, xbex[BL:128, 0:1],
                                Alu.mult, Alu.add)
        nc.gpsimd.tensor_tensor(vb[:, 2:3], vb[:, 1:2], vb[:, 1:2], Alu.mult)
        nc.gpsimd.tensor_tensor(vb[:, 3:4], vb[:, 2:3], vb[:, 1:2], Alu.mult)
        nc.gpsimd.tensor_tensor(vb[:, 4:5], vb[:, 2:3], vb[:, 2:3], Alu.mult)
        nc.gpsimd.tensor_tensor(yt2[:], ty[:], ty[:], Alu.mult)

        # ---- q dots + g dots (DVE, tiny)
        if USE_TTR:
            for j in range(3):
                nc.vector.tensor_tensor_reduce(
                    junkq[:], bas[:], tc[:, C_Q + j * NB:C_Q + (j + 1) * NB],
                    1.0, 0.0, Alu.mult, Alu.add, accum_out=qh[:, j:j + 1])
            for i in range(3):
                nc.vector.tensor_tensor_reduce(
                    junkg[:], vb[:], tc[:, C_G + i * NG:C_G + (i + 1) * NG],
                    1.0, 0.0, Alu.mult, Alu.add, accum_out=gt[:, i:i + 1])
        else:
            jq0 = sb.tile([128, NB], f32, tag="jq0")
            jq1 = sb.tile([128, NB], f32, tag="jq1")
            jq2 = sb.tile([128, NB], f32, tag="jq2")
            for j, jqt in enumerate((jq0, jq1, jq2)):
                nc.vector.tensor_tensor(
                    jqt[:], bas[:], tc[:, C_Q + j * NB:C_Q + (j + 1) * NB],
                    Alu.mult)
                nc.vector.tensor_reduce(qh[:, j:j + 1], jqt[:], Ax.X,
                                        Alu.add)
            jg0 = sb.tile([128, NG], f32, tag="jg0")
            jg1 = sb.tile([128, NG], f32, tag="jg1")
            jg2 = sb.tile([128, NG], f32, tag="jg2")
            for i, jgt in enumerate((jg0, jg1, jg2)):
                nc.vector.tensor_tensor(
                    jgt[:], vb[:], tc[:, C_G + i * NG:C_G + (i + 1) * NG],
                    Alu.mult)
                nc.vector.tensor_reduce(gt[:, i:i + 1], jgt[:], Ax.X,
                                        Alu.add)

        # ---- F2 weights (Scalar exp + gpsimd smalls + 2 DVE recips),
        # emitted early so the final matmul's weights are ready in time
        nc.scalar.activation(eprt[:], fex[:, 1:2], Act.Exp,
                             bias=-dsg['ps_b'], scale=1.0)
        nc.gpsimd.tensor_scalar(numt[:], eprt[:], 1.0, None, Alu.add)
        nc.gpsimd.tensor_scalar(den1[:], eprt[:], 1e-4, 1.0 + 1e-4,
                                Alu.mult, Alu.add)
        nc.gpsimd.tensor_scalar(den0[:], eprt[:], 1.0 + 1e-4, 1e-4,
                                Alu.mult, Alu.add)
        nc.vector.tensor_copy(texs[:], fex[:, 0:1])
        nc.vector.reciprocal(r1[:], den1[:])
        nc.vector.reciprocal(r0[:], den0[:])
        nc.gpsimd.tensor_scalar(f0t[:], texs[:], 0.0, None, Alu.is_equal)
        nc.gpsimd.tensor_scalar(f1t[:], texs[:], -1.0, None,
                                Alu.is_equal)
        nc.gpsimd.tensor_tensor(w0t[:], numt[:], r0[:], Alu.mult)
        nc.gpsimd.tensor_tensor(w1t[:], numt[:], r1[:], Alu.mult)
        nc.gpsimd.tensor_tensor(F2[:, 1:2], f0t[:], w0t[:], Alu.mult)
        nc.gpsimd.tensor_tensor(F2[:, 3:4], f1t[:], w1t[:], Alu.mult)
        nc.gpsimd.tensor_copy(F2[:, 0:1], f0t[:])
        nc.gpsimd.tensor_copy(F2[:, 2:3], f1t[:])

        # ---- adjusted coefs: qx = (q0 - gg*g2 - H3, q1 + 2gg, q2 - H1)
        nc.vector.tensor_tensor(gt2[:, 0:1], gt[:, 1:2], gt[:, 0:1],
                                Alu.mult)                       # gg = H1*g2
        nc.vector.tensor_tensor(qx[:, 2:3], qh[:, 2:3], gt[:, 1:2],
                                Alu.subtract)
        nc.vector.scalar_tensor_tensor(qx[:, 1:2], gt2[:, 0:1], 2.0,
                                       qh[:, 1:2], Alu.mult, Alu.add)
        nc.vector.tensor_tensor(gt2[:, 1:2], gt2[:, 0:1], gt[:, 0:1],
                                Alu.mult)                       # gg*g2
        nc.vector.tensor_tensor(gt2[:, 2:3], qh[:, 0:1], gt2[:, 1:2],
                                Alu.subtract)
        nc.vector.tensor_tensor(qx[:, 0:1], gt2[:, 2:3], gt[:, 2:3],
                                Alu.subtract)

        # ---- qmix (low=raw, up=adj) / qflip (low=adj, up=raw)
        nc.vector.tensor_copy(qmix[0:BL, :], qh[0:BL, :])
        nc.vector.tensor_copy(qmix[BL:128, :], qx[BL:128, :])
        nc.gpsimd.tensor_copy(qflip[0:BL, :], qx[0:BL, :])
        nc.gpsimd.tensor_copy(qflip[BL:128, :], qh[BL:128, :])

        # ---- S pass + col0 fix
        nc.vector.tensor_scalar(S1[:], yt2[:], qmix[:, 2:3], qmix[:, 0:1],
                                Alu.mult, Alu.add)
        nc.vector.scalar_tensor_tensor(R2[:, 0:K], ty[:], qmix[:, 1:2],
                                       S1[:], Alu.mult, Alu.add)
        nc.vector.tensor_scalar(c0a[:], yt2[:, 0:1], qflip[:, 2:3],
                                qflip[:, 0:1], Alu.mult, Alu.add)
        nc.vector.scalar_tensor_tensor(R2[:, 0:1], ty[:, 0:1],
                                       qflip[:, 1:2], c0a[:],
                                       Alu.mult, Alu.add)

        # ---- final masked matmul + out
        nc.tensor.matmul(P[:], F2[:], R2[:], start=True, stop=True)
        nc.vector.tensor_copy(outs[:], P[:])
        nc.sync.dma_start(out_d, outs[:])

    nc.compile()
    return nc


def _host_inputs(inputs, dsg):
    x = np.ascontiguousarray(inputs['x_samples'], dtype=np.float32)
    y = np.ascontiguousarray(inputs['y_samples'], dtype=np.float32)
    ps_w = inputs['ps_w'].astype(np.float32)[:, 0]

    psw2 = np.zeros((2, HC), np.float32)
    psw2[0] = ps_w[0:HC]
    psw2[1, 0:HC - 1] = ps_w[HC:XD - 1]
    psw = np.ascontiguousarray(
        np.broadcast_to(psw2[:, None, :], (2, BL, HC)).reshape(128, HC))

    idx = np.arange(128)
    Mpp = np.zeros((128, 128), np.float32)
    Mpp[idx, idx] = -1.0
    Mpp[idx ^ 64, idx] = -1.0

    consts = np.zeros((128, C_W), np.float32)
    consts[:, 0:128] = Mpp
    for j in range(3):
        consts[0:BL, C_Q + j * NB:C_Q + (j + 1) * NB] = dsg['Rlow'][j]
        consts[BL:128, C_Q + j * NB:C_Q + (j + 1) * NB] = dsg['Rup'][j]
    for i in range(3):
        consts[0:BL, C_G + i * NG:C_G + (i + 1) * NG] = dsg['gcoef_l'][i]
        consts[BL:128, C_G + i * NG:C_G + (i + 1) * NG] = dsg['gcoef_u'][i]

    in_maps = []
    for i in range(NCORES):
        xs = x[i * BL:(i + 1) * BL]                       # [64, 512]
        xt = np.ascontiguousarray(
            xs.reshape(BL, 2, HC).transpose(1, 0, 2).reshape(128, HC))
        ys = y[i * BL:(i + 1) * BL]
        in_maps.append({
            'xt': xt,
            'y2': np.ascontiguousarray(np.vstack([ys, ys])),
            'psw': psw, 'consts': consts,
        })
    return in_maps


def _combine(parts):
    tot = np.zeros((4, K + 1), np.float64)
    for p in parts:
        tot += p.astype(np.float64)
    P0, n0 = tot[0, :K], tot[0, K]
    Q0, r0 = tot[1, :K], tot[1, K]
    P1, n1 = tot[2, :K], tot[2, K]
    Q1, r1 = tot[3, :K], tot[3, K]
    d0 = n0 * (XD - 1)
    d1 = n1 * (XD - 1)
    cmi0 = P0 / d0
    cmi1 = P1 / d1
    dr = 0.5 * ((XD - 1) * cmi0 * (n0 - r0) + Q0) / d0 \
       + 0.5 * ((XD - 1) * cmi1 * (n1 - r1) + Q1) / d1
    cmi_dims = (np.abs(cmi0 + cmi1) / 2.0).astype(np.float32)
    drs = np.abs(dr).astype(np.float32)
    return cmi_dims, drs


def _param_key(inputs, dsg):
    import hashlib
    hsh = hashlib.sha256()
    for k in sorted(inputs):
        if k in ('x_samples', 'y_samples'):
            continue
        hsh.update(k.encode())
        hsh.update(np.ascontiguousarray(inputs[k]).tobytes())
    hsh.update(np.asarray(dsg['Rlow']).tobytes())
    hsh.update(np.asarray(dsg['Rup']).tobytes())
    for g in dsg['gcoef_l'] + dsg['gcoef_u']:
        hsh.update(np.asarray(g).tobytes())
    return hsh.hexdigest()


def kernel(**inputs):
    global _last
    from concourse.bass_utils import run_bass_kernel_spmd

    fc = _fold_consts(inputs)
    dsg = _design(fc, np.asarray(inputs['x_samples']))
    key = _param_key(inputs, dsg)
    if key not in _prog_cache:
        _prog_cache[key] = _build_program(dsg)
    nc = _prog_cache[key]

    in_maps = _host_inputs(inputs, dsg)
    _last = (nc, in_maps)
    res = run_bass_kernel_spmd(nc, in_maps, core_ids=list(range(NCORES)))
    parts = [r['out'] for r in res.results]
    return _combine(parts)


# revision 26
# speedup vs baseline: 1.0784x; 1.0784x over previous
"""Trainium2 Bass kernel for nn_DR_CML (data-parallel over batch, 8 cores).

Math: xm[b,i,j] = x[b,i]*lm_w[j] + lm_b[j], so every loo row is affine in
the scalar s[b,i] = xbar[b] - x[b,i]/xd, and the [B,K,xd-1] diff tensor
collapses to S[b,k] = 511*pos[b,k] + q0[b] + q1[b]*y + q2[b]*y^2 with
q_j[b] = sum_i phi_j(s[b,i]) for smooth scalar functions phi_j.

Device work per core is just: t = -x/512, power sums T1/T2, a propensity
dot, one pair-exchange matmul, then a single [128, 6x12] coefficient dot
producing all six per-row quad coefficients (MAIN family for k>=1 and a
FLIP family for the k=0 column; the positive-branch mu/lv MLPs are
host-fitted as delta-polynomials of v = xbar / s_last and folded into
the per-partition-half coefficient rows).  Everything nonlinear is a
per-call Chebyshev fit on the observed data range (errors ~1e-5; final
f32 rel err ~2e-4 vs the 2e-2 tolerance).

Layout: x repacked [128, 256] (row b in partitions b and b+64); y
duplicated to both halves so the final masked matmul P = F2^T @ R2 sums
halves, masks, and propensity weights in one PE op.  Host sums the 8
[4,33] tiles and applies the closed-form combine.
"""
import math

import numpy as np

B, XD, K, H = 512, 512, 32, 7
NCORES = 8
BL = B // NCORES          # 64 rows per core
HC = XD // 2              # 256 columns after repack
DEGQ = 2                  # phi_j poly degree (power sums T1, T2)
DEGD = 3                  # delta (positive-branch) poly degree
NB2 = 12                  # basis: 1,xb,xb2,T1,T2,xbT1,tr,trxb,xb3,sl,sl2,sl3
LN2 = math.log(2.0)

_prog_cache = {}
_last = None              # (nc, in_maps) from the most recent kernel() call

F2_ON_SCALAR = False       # sigmoid/recip via ScalarE ACTs vs DVE recips


def _fold_consts(p):
    """Fold linear_map + MLP weights into scalar-MLP coefficients (f64)."""
    lm_w = p['lm_w'].astype(np.float64)
    lm_b = p['lm_b'].astype(np.float64)
    c = lm_b * (XD - 1) / XD

    def fold(w1, b1):
        u = lm_w @ w1.astype(np.float64)
        v_base = lm_b @ w1.astype(np.float64) + b1.astype(np.float64)
        v_c = c @ w1.astype(np.float64) + b1.astype(np.float64)
        return u, v_base, v_c

    u_mu, vb_mu, vc_mu = fold(p['mu_w1'], p['mu_b1'])
    u_lv, vb_lv, vc_lv = fold(p['lv_w1'], p['lv_b1'])
    u_mun, _, vc_mun = fold(p['mun_w1'], p['mun_b1'])
    u_lvn, _, vc_lvn = fold(p['lvn_w1'], p['lvn_b1'])

    return {
        'u_mu': u_mu, 'vb_mu': vb_mu, 'vc_mu': vc_mu,
        'u_lv': u_lv, 'vb_lv': vb_lv, 'vc_lv': vc_lv,
        'u_mun': u_mun, 'vc_mun': vc_mun,
        'u_lvn': u_lvn, 'vc_lvn': vc_lvn,
        'w2_mu': p['mu_w2'][:, 0].astype(np.float64),
        'w2_lv': p['lv_w2'][:, 0].astype(np.float64),
        'w2_mun': p['mun_w2'][:, 0].astype(np.float64),
        'w2_lvn': p['lvn_w2'][:, 0].astype(np.float64),
        'b2_mu': float(p['mu_b2'][0]), 'b2_lv': float(p['lv_b2'][0]),
        'b2_mun': float(p['mun_b2'][0]), 'b2_lvn': float(p['lvn_b2'][0]),
        'ps_b': float(p['ps_b'][0]),
    }


def _fit_poly(f, lo, hi, deg):
    if hi - lo < 1e-12:
        hi = lo + 1e-6
    c = np.polynomial.chebyshev.Chebyshev.interpolate(f, deg, domain=[lo, hi])
    out = np.zeros(deg + 1)
    cc = c.convert(kind=np.polynomial.Polynomial).coef
    out[:len(cc)] = cc
    return out


def _design(fc, x):
    """Per-call poly fits folded into one [128, 6, NB2] coef tensor."""
    x64 = x.astype(np.float64)
    xbar = x64.mean(1)
    s = xbar[:, None] - x64[:, :XD - 1] / XD
    smin, smax = float(s.min()), float(s.max())

    def mlp(v, u, vb, w2, b2):
        return np.maximum(np.multiply.outer(v, u) + vb, 0.0) @ w2 + b2

    def phi(sv, idx):
        mun = mlp(sv, fc['u_mun'], fc['vc_mun'], fc['w2_mun'], fc['b2_mun'])
        lvn = np.tanh(mlp(sv, fc['u_lvn'], fc['vc_lvn'], fc['w2_lvn'],
                          fc['b2_lvn']))
        ev = np.exp(-lvn - LN2)
        return [ev * mun * mun + 0.5 * lvn, -2.0 * ev * mun, ev][idx]

    qc = [_fit_poly(lambda t: phi(t, j), smin, smax, DEGQ) for j in range(3)]

    def qrows(upper):
        R = np.zeros((3, NB2))
        for j in range(3):
            for d in range(DEGQ + 1):
                c = qc[j][d]
                if c == 0.0:
                    continue
                for m in range(d + 1):
                    k = d - m
                    w = c * math.comb(d, m)
                    if k == 0:
                        R[j, m] += w * float(HC)       # Traw_0 = 256
                    elif k == 1:
                        R[j, [3, 5][m]] += w           # T1, xb*T1
                    else:
                        R[j, 4] += w                   # T2
                if upper:  # exclude the i=511 (treat) slot
                    for m in range(d + 1):
                        k = d - m
                        w = -c * math.comb(d, m) * (-1.0 / XD) ** k
                        if k == 0:
                            R[j, m] += w
                        else:
                            R[j, 6 + m] += w           # tr * xb^m
        return R

    slast = s[:, -1]
    ranges = {'low': (float(xbar.min()), float(xbar.max())),
              'up': (float(slast.min()), float(slast.max()))}

    def gfn(vb_mu, vb_lv):
        mu = lambda v: mlp(v, fc['u_mu'], vb_mu, fc['w2_mu'], fc['b2_mu'])
        lv = lambda v: np.tanh(mlp(v, fc['u_lv'], vb_lv, fc['w2_lv'],
                                   fc['b2_lv']))
        H1 = lambda v: (XD - 1) * np.exp(-lv(v)) * 0.5
        H3 = lambda v: (XD - 1) * lv(v) * 0.5
        return (lambda v: -H1(v) * mu(v) ** 2 - H3(v),
                lambda v: 2.0 * H1(v) * mu(v),
                lambda v: -H1(v))

    dco = {}
    for half, (vm, vl) in (('low', (fc['vb_mu'], fc['vb_lv'])),
                           ('up', (fc['vc_mu'], fc['vc_lv']))):
        lo, hi = ranges[half]
        dco[half] = [_fit_poly(g, lo, hi, DEGD) for g in gfn(vm, vl)]

    def drow(half, j):
        r = np.zeros(NB2)
        cols = [0, 1, 2, 8] if half == 'low' else [0, 9, 10, 11]
        for d in range(DEGD + 1):
            r[cols[d]] += dco[half][j][d]
        return r

    Ql, Qu = qrows(False), qrows(True)
    CM = np.zeros((128, 6, NB2))
    for j in range(3):
        CM[0:BL, j] = Ql[j]
        CM[BL:128, j] = Qu[j] + drow('up', j)      # MAIN (k>=1 columns)
        CM[0:BL, 3 + j] = Ql[j] + drow('low', j)   # FLIP (k=0 column)
        CM[BL:128, 3 + j] = Qu[j]
    return {'CM': CM, 'ps_b': fc['ps_b']}


# ---- consts tensor layout
C_M = 0                    # cols 0:128    M'' = -M pair-exchange matrix
C_C = 128                  # cols 128:200  [128, 6*NB2] coefficient rows
C_W = C_C + 6 * NB2


def _build_program(dsg):
    from contextlib import ExitStack
    import concourse.tile as tile
    from concourse import bacc, mybir

    f32 = mybir.dt.float32
    Alu = mybir.AluOpType
    Act = mybir.ActivationFunctionType
    Ax = mybir.AxisListType

    nc = bacc.Bacc("TRN2", target_bir_lowering=False, debug=False,
                   num_devices=NCORES)

    xt_d = nc.dram_tensor("xt", [2 * BL, HC], f32, kind="ExternalInput").ap()
    y2_d = nc.dram_tensor("y2", [2 * BL, K], f32, kind="ExternalInput").ap()
    psw_d = nc.dram_tensor("psw", [2 * BL, HC], f32,
                           kind="ExternalInput").ap()
    tc_d = nc.dram_tensor("consts", [128, C_W], f32,
                          kind="ExternalInput").ap()
    out_d = nc.dram_tensor("out", [4, K + 1], f32, kind="ExternalOutput").ap()

    with tile.TileContext(nc) as tcx, ExitStack() as ctx:
        sb = ctx.enter_context(tcx.tile_pool(name="sb", bufs=1))
        ps = ctx.enter_context(tcx.tile_pool(name="ps", bufs=1, space="PSUM"))

        # ---- DMAs: xt+y2 on sync, consts on scalar, psw on gpsimd
        tx = sb.tile([128, HC], f32, tag="tx")
        nc.sync.dma_start(tx[:], xt_d)
        tc = sb.tile([128, C_W], f32, tag="tc")
        nc.scalar.dma_start(tc[:], tc_d)
        tpsw = sb.tile([128, HC], f32, tag="tpsw")
        nc.gpsimd.dma_start(tpsw[:], psw_d)
        ty = sb.tile([128, K], f32, tag="ty")
        nc.sync.dma_start(ty[:], y2_d)

        # hoist the ACT table load before data arrives
        warm = sb.tile([1, 1], f32, tag="warm")
        nc.scalar.activation(warm[:], nc.const_aps.tensor(0.0, (1, 1)),
                             Act.Sigmoid, bias=0.0, scale=1.0)

        Mpp = tc[:, C_M:C_M + 128]

        # ---- tiles
        bas = sb.tile([128, NB2], f32, tag="bas")
        st = sb.tile([128, 2], f32, tag="st")      # [tr-stage | rp]
        q6 = sb.tile([128, 6], f32, tag="q6")
        rep = sb.tile([128, 6 * NB2], f32, tag="rep")
        yt2 = sb.tile([128, K], f32, tag="yt2")
        R2 = sb.tile([128, K + 1], f32, tag="R2")
        F2 = sb.tile([128, 4], f32, tag="F2")
        S1 = sb.tile([128, K - 1], f32, tag="S1")
        c0a = sb.tile([128, 1], f32, tag="c0a")
        propt = sb.tile([128, 1], f32, tag="propt")
        den0 = sb.tile([128, 1], f32, tag="den0")
        den1 = sb.tile([128, 1], f32, tag="den1")
        r0 = sb.tile([128, 1], f32, tag="r0")
        r1 = sb.tile([128, 1], f32, tag="r1")
        f0t = sb.tile([128, 1], f32, tag="f0t")
        f1t = sb.tile([128, 1], f32, tag="f1t")
        texs = sb.tile([128, 1], f32, tag="texs")
        t = sb.tile([128, HC], f32, tag="t")
        junkp = sb.tile([128, HC], f32, tag="junkp")
        junk2 = sb.tile([128, HC], f32, tag="junk2")
        outs = sb.tile([4, K + 1], f32, tag="outs")

        xbex = ps.tile([128, 1], f32, tag="xbex")
        fex = ps.tile([128, 2], f32, tag="fex")
        P = ps.tile([4, K + 1], f32, tag="P")

        # ---- gpsimd: early memsets (no data deps)
        nc.gpsimd.memset(bas[:, 0:1], 1.0)
        nc.gpsimd.memset(bas[0:BL, 6:8], 0.0)
        nc.gpsimd.memset(bas[0:BL, 9:12], 0.0)
        nc.gpsimd.memset(R2[0:BL, K:K + 1], 1.0)
        nc.gpsimd.memset(R2[BL:128, K:K + 1], 0.0)
        nc.gpsimd.memset(st[0:BL, 0:1], 0.0)

        # ---- gpsimd: data-dependent helpers
        nc.gpsimd.tensor_copy(st[BL:128, 0:1], tx[BL:128, HC - 1:HC])
        nc.gpsimd.tensor_copy(bas[BL:128, 6:7], tx[BL:128, HC - 1:HC])

        # ---- DVE spine
        nc.vector.tensor_scalar(t[:], tx[:], -1.0 / XD, None, Alu.mult)
        nc.vector.tensor_reduce(bas[:, 3:4], t[:], Ax.X, Alu.add)

        nc.gpsimd.tensor_tensor(junk2[:], t[:], t[:], Alu.mult)
        nc.gpsimd.tensor_tensor(junkp[:], tx[:], tpsw[:], Alu.mult)
        nc.gpsimd.tensor_tensor(yt2[:], ty[:], ty[:], Alu.mult)

        nc.vector.tensor_reduce(bas[:, 4:5], junk2[:], Ax.X, Alu.add)
        nc.vector.tensor_reduce(st[:, 1:2], junkp[:], Ax.X, Alu.add)

        # ---- PE: exchanges
        nc.tensor.matmul(xbex[:], Mpp, bas[:, 3:4], start=True, stop=True)
        nc.tensor.matmul(fex[:], Mpp, st[:], start=True, stop=True)

        # ---- basis completion (DVE tinies)
        nc.vector.tensor_copy(bas[:, 1:2], xbex[:])
        nc.vector.tensor_tensor(bas[:, 2:3], bas[:, 1:2], bas[:, 1:2],
                                Alu.mult)
        nc.vector.tensor_tensor(bas[:, 5:6], bas[:, 1:2], bas[:, 3:4],
                                Alu.mult)
        nc.vector.tensor_tensor(bas[:, 7:8], bas[:, 6:7], bas[:, 1:2],
                                Alu.mult)
        nc.vector.tensor_tensor(bas[:, 8:9], bas[:, 2:3], bas[:, 1:2],
                                Alu.mult)
        nc.vector.tensor_scalar(bas[BL:128, 9:10], tx[BL:128, HC - 2:HC - 1],
                                -1.0 / XD, xbex[BL:128, 0:1],
                                Alu.mult, Alu.add)
        nc.vector.tensor_tensor(bas[BL:128, 10:11], bas[BL:128, 9:10],
                                bas[BL:128, 9:10], Alu.mult)
        nc.vector.tensor_tensor(bas[BL:128, 11:12], bas[BL:128, 10:11],
                                bas[BL:128, 9:10], Alu.mult)

        # ---- one dot: all six per-row quad coefficients
        bas_bc = bas[:].unsqueeze(1).broadcast_to([128, 6, NB2])
        cm3 = tc[:, C_C:C_C + 6 * NB2].rearrange("p (g f) -> p g f", g=6)
        rep3 = rep[:].rearrange("p (g f) -> p g f", g=6)
        nc.vector.tensor_tensor(rep3, bas_bc, cm3, Alu.mult)
        nc.vector.tensor_reduce(q6[:], rep3, Ax.X, Alu.add)

        # ---- F2 weights
        nc.scalar.activation(propt[:], fex[:, 1:2], Act.Sigmoid,
                             bias=dsg['ps_b'], scale=-1.0)
        if F2_ON_SCALAR:
            nc.scalar.activation(den1[:], propt[:], Act.Identity,
                                 bias=1e-4, scale=1.0)
            nc.scalar.activation(r1[:], den1[:], Act.Reciprocal,
                                 bias=0.0, scale=1.0)
            nc.scalar.activation(den0[:], propt[:], Act.Identity,
                                 bias=1.0 + 1e-4, scale=-1.0)
            nc.scalar.activation(r0[:], den0[:], Act.Reciprocal,
                                 bias=0.0, scale=1.0)
        else:
            nc.gpsimd.tensor_scalar(den1[:], propt[:], 1e-4, None, Alu.add)
            nc.gpsimd.tensor_scalar(den0[:], propt[:], -1.0, 1.0 + 1e-4,
                                    Alu.mult, Alu.add)
            nc.vector.reciprocal(r1[:], den1[:])
            nc.vector.reciprocal(r0[:], den0[:])
        nc.scalar.copy(texs[:], fex[:, 0:1])
        nc.gpsimd.tensor_scalar(f0t[:], texs[:], 0.0, None, Alu.is_equal)
        nc.gpsimd.tensor_scalar(f1t[:], texs[:], -1.0, None, Alu.is_equal)
        nc.gpsimd.tensor_tensor(F2[:, 1:2], f0t[:], r0[:], Alu.mult)
        nc.gpsimd.tensor_tensor(F2[:, 3:4], f1t[:], r1[:], Alu.mult)
        nc.gpsimd.tensor_copy(F2[:, 0:1], f0t[:])
        nc.gpsimd.tensor_copy(F2[:, 2:3], f1t[:])

        # ---- S pass (MAIN cols 0-2 for k>=1, FLIP cols 3-5 for k=0)
        nc.vector.tensor_scalar(S1[:], yt2[:, 1:K], q6[:, 2:3], q6[:, 0:1],
                                Alu.mult, Alu.add)
        nc.vector.scalar_tensor_tensor(R2[:, 1:K], ty[:, 1:K], q6[:, 1:2],
                                       S1[:], Alu.mult, Alu.add)
        nc.vector.tensor_scalar(c0a[:], yt2[:, 0:1], q6[:, 5:6], q6[:, 3:4],
                                Alu.mult, Alu.add)
        nc.vector.scalar_tensor_tensor(R2[:, 0:1], ty[:, 0:1], q6[:, 4:5],
                                       c0a[:], Alu.mult, Alu.add)

        # ---- final masked matmul + out
        nc.tensor.matmul(P[:], F2[:], R2[:], start=True, stop=True)
        nc.vector.tensor_copy(outs[:], P[:])
        nc.sync.dma_start(out_d, outs[:])

    nc.compile()
    return nc


def _host_inputs(inputs, dsg):
    x = np.ascontiguousarray(inputs['x_samples'], dtype=np.float32)
    y = np.ascontiguousarray(inputs['y_samples'], dtype=np.float32)
    ps_w = inputs['ps_w'].astype(np.float32)[:, 0]

    psw2 = np.zeros((2, HC), np.float32)
    psw2[0] = ps_w[0:HC]
    psw2[1, 0:HC - 1] = ps_w[HC:XD - 1]
    psw = np.ascontiguousarray(
        np.broadcast_to(psw2[:, None, :], (2, BL, HC)).reshape(128, HC))

    idx = np.arange(128)
    Mpp = np.zeros((128, 128), np.float32)
    Mpp[idx, idx] = -1.0
    Mpp[idx ^ 64, idx] = -1.0

    consts = np.zeros((128, C_W), np.float32)
    consts[:, 0:128] = Mpp
    consts[:, C_C:C_C + 6 * NB2] = dsg['CM'].reshape(128, 6 * NB2)

    in_maps = []
    for i in range(NCORES):
        xs = x[i * BL:(i + 1) * BL]                       # [64, 512]
        xt = np.ascontiguousarray(
            xs.reshape(BL, 2, HC).transpose(1, 0, 2).reshape(128, HC))
        ys = y[i * BL:(i + 1) * BL]
        in_maps.append({
            'xt': xt,
            'y2': np.ascontiguousarray(np.vstack([ys, ys])),
            'psw': psw, 'consts': consts,
        })
    return in_maps


def _combine(parts):
    tot = np.zeros((4, K + 1), np.float64)
    for p in parts:
        tot += p.astype(np.float64)
    P0, n0 = tot[0, :K], tot[0, K]
    Q0, r0 = tot[1, :K], tot[1, K]
    P1, n1 = tot[2, :K], tot[2, K]
    Q1, r1 = tot[3, :K], tot[3, K]
    d0 = n0 * (XD - 1)
    d1 = n1 * (XD - 1)
    cmi0 = P0 / d0
    cmi1 = P1 / d1
    dr = 0.5 * ((XD - 1) * cmi0 * (n0 - r0) + Q0) / d0 \
       + 0.5 * ((XD - 1) * cmi1 * (n1 - r1) + Q1) / d1
    cmi_dims = (np.abs(cmi0 + cmi1) / 2.0).astype(np.float32)
    drs = np.abs(dr).astype(np.float32)
    return cmi_dims, drs


def _param_key(inputs, dsg):
    import hashlib
    hsh = hashlib.sha256()
    for k in sorted(inputs):
        if k in ('x_samples', 'y_samples'):
            continue
        hsh.update(k.encode())
        hsh.update(np.ascontiguousarray(inputs[k]).tobytes())
    hsh.update(np.asarray(dsg['CM']).tobytes())
    return hsh.hexdigest()


def kernel(**inputs):
    global _last
    from concourse.bass_utils import run_bass_kernel_spmd

    fc = _fold_consts(inputs)
    dsg = _design(fc, np.asarray(inputs['x_samples']))
    key = _param_key(inputs, dsg)
    if key not in _prog_cache:
        _prog_cache[key] = _build_program(dsg)
    nc = _prog_cache[key]

    in_maps = _host_inputs(inputs, dsg)
    _last = (nc, in_maps)
    res = run_bass_kernel_spmd(nc, in_maps, core_ids=list(range(NCORES)))
    parts = [r['out'] for r in res.results]
    return _combine(parts)
